# revision 1
# baseline (speedup 1.0000x reference)
"""Trainium2 Bass kernel for the YOLO-style DetectionLayer loss.

Strategy (data parallel over batch, 4 samples/core on 8 cores):
The six losses depend on x only at the 80 ground-truth cells per core
(4 samples x 20 GTs), plus a closed-form constant for the empty-cell part
of the class loss.  x is fed channel-last and padded to 256 channels per
cell (pure layout permutation done while sharding) so each GT cell's
channels are one aligned 1KB run in DRAM.  Each core:
  1. loads a tiny [128,8] tensor (y_true + per-partition index constants)
     and, in parallel, a [128,264] constants tensor (anchors, masks, ...),
  2. computes cell indices from y_true in a short all-vector chain and
     gathers the 80 GT cells' channels with one 80-descriptor indirect DMA,
  3. while the gather flies, computes every x-independent quantity,
  4. computes per-anchor IoU vs the GT box (center-form overlap: no
     corner boxes needed), picks the best anchor by value equality,
     selects that anchor's values with one multiply + strided reduce,
  5. computes the per-GT loss columns using fused ops
     (scalar_tensor_tensor, tensor_tensor_reduce with accum_out,
     activation with accum_out) and kills duplicate (cell, anchor)
     entries via a PE transpose + masked max (last write wins, matching
     jax scatter semantics),
  6. reduces the 80 loss rows with a ones-free matmul (weights = -live),
Each core returns 8 partial sums; the host combines (negate, x25 for conf,
and the closed-form empty-cell class constant via the live-count column).

All arithmetic on tensor values runs on device; the host only does layout
permutation/replication while sharding and combines the 8x8 output scalars.
"""

import numpy as np

import concourse.bacc as bacc
import concourse.bass as bass
import concourse.mybir as mybir
import concourse.tile as tile
from concourse.bass import IndirectOffsetOnAxis
from concourse.bass_utils import run_bass_kernel_spmd

# Problem shape (hardcoded per harness contract).
BS, GS, NA, NCLS, NGT = 32, 52, 3, 80, 20
NCORES = 8
BPC = BS // NCORES          # samples per core
P = 128
NGTC = BPC * NGT            # ground truths per core (80) -- all rows valid
CH = 5 + NCLS               # channels per anchor (85)
NCH = NA * CH               # 255 real channels
CHP = 256                   # padded channels per cell (1KB aligned)
PLANE = GS * GS             # 2704 cells
XF2 = BPC * PLANE * CHP
LN80 = float(np.log(np.float64(NCLS)))
CYW = 8                     # small per-GT input width
CBW = 360                   # big constants width

F32 = mybir.dt.float32
I32 = mybir.dt.int32
A = mybir.AluOpType
AF = mybir.ActivationFunctionType
AX = mybir.AxisListType


def _patch_act_tables():
    """Steer the act-table chooser so Exp and Ln both resolve to the one
    set that contains both (`natural_log_exp_and_others`) -> exactly one
    ACT_TABLE_LOAD in the kernel."""
    from concourse import hw_specs, bacc as bacc_mod
    orig = hw_specs.get_activation_tables

    def patched(arch):
        t = {k: set(v) for k, v in orig(arch).items()}
        if "natural_log_exp_and_others" in t:
            for name in t:
                if name != "natural_log_exp_and_others":
                    t[name] = t[name] - {AF.Exp, AF.Ln}
        return t

    hw_specs.get_activation_tables = patched
    bacc_mod.get_activation_tables = patched
    return orig


def _unpatch_act_tables(orig):
    from concourse import hw_specs, bacc as bacc_mod
    hw_specs.get_activation_tables = orig
    bacc_mod.get_activation_tables = orig


def _build(dbg=False):
    nc = bacc.Bacc("TRN2", target_bir_lowering=False, debug=False,
                   num_devices=NCORES)
    xf = nc.dram_tensor("xf", [XF2, 1], F32, kind="ExternalInput")
    cy_d = nc.dram_tensor("cy", [P, CYW], F32, kind="ExternalInput")
    cb_d = nc.dram_tensor("cb", [P, CBW], F32, kind="ExternalInput")
    out_d = nc.dram_tensor("out", [1, 8], F32, kind="ExternalOutput")
    if dbg:
        dbg_d = nc.dram_tensor("dbg", [P, 32], F32, kind="ExternalOutput")

    v, s, g, te, sy = nc.vector, nc.scalar, nc.gpsimd, nc.tensor, nc.sync
    N = NGTC

    with tile.TileContext(nc) as tc:
        with tc.tile_pool(name="sb", bufs=1) as sb, \
             tc.tile_pool(name="ps", bufs=1, space="PSUM") as ps:
            cy = sb.tile([P, CYW], F32)
            cb = sb.tile([P, CBW], F32)
            sy.dma_start(out=cy[:], in_=cy_d[:])
            sy.dma_start(out=cb[:], in_=cb_d[:])
            ytx = cy[:, 0:4]          # y_true xc, yc, w, h (pad 0.5)
            lblr = cy[:, 4:5]         # y_true class (float, exact int)
            bb256 = cy[:, 5:6]        # b_local * PLANE * 256
            b3pl = cy[:, 6:7]         # b_local * 3 * PLANE
            valid = cy[:, 7:8]        # 1.0 for p < 80
            ancr = cb[:, 0:6]         # anchors replicated, a-major (w,h)
            k15 = cb[:, 6:21]         # sigmoid-vs-exp selector 1,1,0,0,1 x3
            iocf = cb[:, 21:101]      # iota 0..79
            pl3 = cb[:, 101:104]      # 0, PLANE, 2*PLANE
            ident = cb[:, 104:232]    # eye(128)
            upper = cb[:, 232:360]    # 1.0 where p < col < 80, p < 80

            zb = sb.tile([P, 2], F32)
            v.memset(zb[:], 0.0)
            gt = sb.tile([P, CHP], F32)
            v.memset(gt[:], 0.0)      # defines pad rows before the gather
            tg5 = sb.tile([P, 5], F32)
            v.memset(tg5[:, 2:4], 0.5)
            loss = sb.tile([P, 8], F32)
            v.memset(loss[:, 6:7], 1.0)   # live-count column
            v.memset(loss[:, 7:8], 0.0)
            # dummy ACT with no data deps: triggers the (single) activation
            # table load at kernel start, overlapping the startup window.
            warm = sb.tile([P, 1], F32)
            s.activation(out=warm[:], in_=zb[:, 0:1], func=AF.Exp,
                         bias=zb[:, 1:2], scale=1.0)

            # ---- grid cell indices (critical path to the gather) ----
            # robust floor for either i32-cast mode: c = cast(v); c -= (c > v)
            gxy = sb.tile([P, 4], F32)          # gx, gy, gw, gh grid units
            v.tensor_scalar_mul(out=gxy[:], in0=ytx, scalar1=float(GS))
            ci = sb.tile([P, 2], I32)
            v.tensor_copy(out=ci[:], in_=gxy[:, 0:2])
            cf = sb.tile([P, 2], F32)
            v.tensor_copy(out=cf[:], in_=ci[:])
            fx = sb.tile([P, 2], F32)
            v.tensor_tensor(out=fx[:], in0=cf[:], in1=gxy[:, 0:2], op=A.is_gt)
            ijf = sb.tile([P, 2], F32)          # gi, gj floored, as f32
            v.tensor_sub(ijf[:], cf[:], fx[:])
            cell0 = sb.tile([P, 1], F32)        # gj*GS + gi
            v.tensor_scalar(out=cell0[:], in0=ijf[:, 1:2], scalar1=float(GS),
                            scalar2=ijf[:, 0:1], op0=A.mult, op1=A.add)
            idx1f = sb.tile([P, 1], F32)
            v.tensor_scalar(out=idx1f[:], in0=cell0[:], scalar1=float(CHP),
                            scalar2=bb256, op0=A.mult, op1=A.add)
            idx1 = sb.tile([P, 1], I32)
            v.tensor_copy(out=idx1[:], in_=idx1f[:])

            # ---- gather the 80 GT cells' channels (one indirect DMA) ----
            g.indirect_dma_start(
                out=gt[0:P, 0:CHP], out_offset=None, in_=xf[:],
                in_offset=IndirectOffsetOnAxis(ap=idx1[0:P, 0:1], axis=0))
            p1v = gt[:, 0:NCH].rearrange("p (a c) -> p a c", a=NA)

            # ---- x-independent work, emitted here to fill gather latency --
            v.tensor_sub(tg5[:, 0:2], gxy[:, 0:2], ijf[:])   # frac targets
            gwhh = sb.tile([P, 2], F32)         # gw/2, gh/2
            v.tensor_scalar_mul(out=gwhh[:], in0=gxy[:, 2:4], scalar1=0.5)
            rgwh = sb.tile([P, 2], F32)         # 1/gw, 1/gh
            v.reciprocal(out=rgwh[:], in_=gxy[:, 2:4])
            jk1 = sb.tile([P, 1], F32)
            areag1 = sb.tile([P, 1], F32)       # gw*gh + 1e-16
            v.tensor_tensor(out=jk1[:], in0=gxy[:, 2:3], in1=gxy[:, 3:4],
                            op=A.mult)
            v.tensor_scalar(out=areag1[:], in0=jk1[:], scalar1=1e-16,
                            scalar2=None, op0=A.add)
            q = sb.tile([P, 1], F32)            # b*3*PLANE + cell
            v.tensor_scalar(out=q[:], in0=cell0[:], scalar1=b3pl,
                            scalar2=None, op0=A.add)
            oh = sb.tile([P, NCLS], F32)        # one-hot of class label
            v.tensor_scalar(out=oh[:], in0=iocf, scalar1=lblr,
                            scalar2=None, op0=A.is_equal)
            sa = sb.tile([P, 6], F32)           # anchors / stride, a-major
            v.tensor_scalar_mul(out=sa[:], in0=ancr, scalar1=1.0 / (416 // GS))
            gwhhb = gwhh[:, 0:2].rearrange("p (o c) -> p o c", o=1).to_broadcast((P, NA, 2))
            gtxyb = tg5[:, 0:2].rearrange("p (o c) -> p o c", o=1).to_broadcast((P, NA, 2))

            # ---- x-dependent chain ----
            # r = 1/(exp(-v)+k): k=1 -> sigmoid(tx,ty,conf); k=0 -> exp(tw,th)
            ex = sb.tile([P, 15], F32)
            exv = ex[:, 0:15].rearrange("p (a c) -> p a c", a=NA)
            s.activation(out=exv, in_=p1v[:, :, 0:5], func=AF.Exp,
                         bias=zb[:, 0:1], scale=-1.0)
            v.tensor_add(ex[:], ex[:], k15)
            r15 = sb.tile([P, 15], F32)
            v.reciprocal(out=r15[:], in_=ex[:])
            rv = r15[:, 0:15].rearrange("p (a c) -> p a c", a=NA)
            sav = sa[:, 0:6].rearrange("p (a c) -> p a c", a=NA)
            # in place: cols 2,3 per anchor become bw, bh (grid units)
            v.tensor_tensor(out=rv[:, :, 2:4], in0=rv[:, :, 2:4], in1=sav,
                            op=A.mult)

            # ---- IoU via center-form overlap ----
            dx = sb.tile([P, 6], F32)
            dxv = dx[:, 0:6].rearrange("p (a c) -> p a c", a=NA)
            v.tensor_tensor(out=dxv, in0=rv[:, :, 0:2], in1=gtxyb, op=A.subtract)
            ext0 = sb.tile([P, 6], F32)
            ext0v = ext0[:, 0:6].rearrange("p (a c) -> p a c", a=NA)
            bh6 = sb.tile([P, 6], F32)
            bh6v = bh6[:, 0:6].rearrange("p (a c) -> p a c", a=NA)
            v.tensor_scalar_mul(out=bh6v, in0=rv[:, :, 2:4], scalar1=0.5)
            v.tensor_tensor(out=ext0v, in0=bh6v, in1=gwhhb, op=A.add)
            emin = sb.tile([P, 6], F32)
            v.tensor_sub(emin[:], ext0[:], dx[:])
            epl = sb.tile([P, 6], F32)
            v.tensor_add(epl[:], ext0[:], dx[:])
            extn = sb.tile([P, 6], F32)
            extnv = extn[:, 0:6].rearrange("p (a c) -> p a c", a=NA)
            v.tensor_tensor(out=extn[:], in0=emin[:], in1=epl[:], op=A.min)
            # containment: overlap = min((a+b)-|d|, bw, gw)
            v.tensor_tensor(out=extnv, in0=extnv, in1=rv[:, :, 2:4], op=A.min)
            gwhb = gxy[:, 2:4].rearrange("p (o c) -> p o c", o=1).to_broadcast((P, NA, 2))
            v.tensor_tensor(out=extnv, in0=extnv, in1=gwhb, op=A.min)
            v.tensor_scalar_max(out=extn[:], in0=extn[:], scalar1=0.0)
            extv = extn[:, 0:6].rearrange("p (a c) -> p a c", a=NA)
            inter = sb.tile([P, 3], F32)
            interv = inter[:, 0:3].rearrange("p (a o) -> p a o", o=1)
            v.tensor_tensor(out=interv, in0=extv[:, :, 0:1], in1=extv[:, :, 1:2],
                            op=A.mult)
            areab = sb.tile([P, 3], F32)
            areabv = areab[:, 0:3].rearrange("p (a o) -> p a o", o=1)
            v.tensor_tensor(out=areabv, in0=rv[:, :, 2:3], in1=rv[:, :, 3:4],
                            op=A.mult)
            union = sb.tile([P, 3], F32)
            v.tensor_scalar(out=union[:], in0=areab[:], scalar1=areag1[:, 0:1],
                            scalar2=None, op0=A.add)
            v.tensor_sub(union[:], union[:], inter[:])
            recu = sb.tile([P, 3], F32)
            v.reciprocal(out=recu[:], in_=union[:])
            iou = sb.tile([P, 3], F32)
            v.tensor_mul(iou[:], inter[:], recu[:])

            # ---- best anchor: value-equality one-hot (no exact ties) ----
            v.tensor_reduce(out=tg5[:, 4:5], in_=iou[:], axis=AX.X, op=A.max)
            w3 = sb.tile([P, 3], F32)
            v.tensor_scalar(out=w3[:], in0=iou[:], scalar1=tg5[:, 4:5],
                            scalar2=None, op0=A.is_equal)

            # ---- dedup key: lin = b*3*PLANE + a*PLANE + cell ----
            jk3 = sb.tile([P, 3], F32)
            lin = sb.tile([P, 1], F32)
            v.tensor_tensor(out=jk3[:], in0=w3[:], in1=pl3, op=A.mult)
            v.tensor_reduce(out=lin[:], in_=jk3[:], axis=AX.X, op=A.add)
            v.tensor_scalar(out=lin[:], in0=lin[:], scalar1=q[:, 0:1],
                            scalar2=None, op0=A.add)
            tp = ps.tile([P, P], F32)
            te.transpose(out=tp[:],
                         in_=lin[:, 0:1].to_broadcast((P, P)),
                         identity=ident)

            # ---- select best-anchor values (overlaps the PE transpose) ----
            # c-major product so the innermost (anchor) stride is nonzero
            w5b = w3[:, 0:3].rearrange("p (o a) -> p o a", o=1).to_broadcast((P, 5, NA))
            selw = sb.tile([P, 15], F32)
            selwv = selw[:, 0:15].rearrange("p (c a) -> p c a", a=NA)
            v.tensor_tensor(out=selwv, in0=r15[:, 0:15].rearrange(
                "p (a c) -> p c a", a=NA), in1=w5b, op=A.mult)
            sel5 = sb.tile([P, 5], F32)        # sigx, sigy, bw, bh, sigconf
            v.tensor_reduce(out=sel5[:], in_=selwv, axis=AX.X, op=A.add)
            clw = sb.tile([P, NA * NCLS], F32)
            clwv = clw[:, 0:NA * NCLS].rearrange("p (j a) -> p j a", a=NA)
            wcb = w3[:, 0:3].rearrange("p (o a) -> p o a", o=1).to_broadcast((P, NCLS, NA))
            cls_v = p1v[:, :, 5:CH].rearrange("p a j -> p j a")
            v.tensor_tensor(out=clwv, in0=cls_v, in1=wcb, op=A.mult)
            cl = sb.tile([P, NCLS], F32)
            v.tensor_reduce(out=cl[:], in_=clwv, axis=AX.X, op=A.add)

            # ---- loss columns 0..4: (sel - target)^2 ----
            dall = sb.tile([P, 5], F32)
            v.tensor_sub(dall[:], sel5[:], tg5[:])
            v.tensor_mul(loss[:, 0:5], dall[:], dall[:])
            lnin = sb.tile([P, 3], F32)        # bw/gw, bh/gh, sum(exp)
            v.tensor_mul(lnin[:, 0:2], sel5[:, 2:4], rgwh[:])

            # ---- cross entropy of the selected logits ----
            mxp = sb.tile([P, 1], F32)
            v.tensor_reduce(out=mxp[:], in_=cl[:], axis=AX.X, op=A.max)
            nm = sb.tile([P, 1], F32)          # -max(cl)
            v.tensor_scalar_mul(out=nm[:], in0=mxp[:], scalar1=-1.0)
            jk80 = sb.tile([P, NCLS], F32)
            logit = sb.tile([P, 1], F32)
            v.tensor_tensor(out=jk80[:], in0=cl[:], in1=oh[:], op=A.mult)
            v.tensor_reduce(out=logit[:], in_=jk80[:], axis=AX.X, op=A.add)
            ez = sb.tile([P, NCLS], F32)
            s.activation(out=ez[:], in_=cl[:], func=AF.Exp, bias=nm[:, 0:1],
                         scale=1.0)
            v.tensor_reduce(out=lnin[:, 2:3], in_=ez[:], axis=AX.X, op=A.add)
            lnout = sb.tile([P, 3], F32)       # ln(rw), ln(rh), lse
            s.activation(out=lnout[:], in_=lnin[:], func=AF.Ln,
                         bias=zb[:, 0:1], scale=1.0)
            # lw = (tw - ln(gw/wa))^2 = ln(bw/gw)^2 ; same for h
            v.tensor_mul(loss[:, 2:4], lnout[:, 0:2], lnout[:, 0:2])
            # ce = lse_ln - (logit + nm)  (-ln80 handled via live count col)
            lgnm = sb.tile([P, 1], F32)
            v.tensor_add(lgnm[:], logit[:], nm[:])
            v.tensor_scalar(out=loss[:, 5:6], in0=lnout[:, 2:3],
                            scalar1=lgnm[:, 0:1], scalar2=None,
                            op0=A.subtract)

            # ---- dedup: kill earlier duplicates (last write wins) ----
            eqm = sb.tile([P, P], F32)
            v.tensor_scalar(out=eqm[:], in0=tp[:],
                            scalar1=lin[:, 0:1], scalar2=None,
                            op0=A.is_equal)
            jkn = sb.tile([P, P], F32)
            kil = sb.tile([P, 1], F32)
            v.tensor_tensor(out=jkn[:], in0=eqm[:], in1=upper, op=A.mult)
            v.tensor_reduce(out=kil[:], in_=jkn[:], axis=AX.X, op=A.max)
            neglive = sb.tile([P, 1], F32)     # kil - valid = -(live)
            v.tensor_scalar(out=neglive[:], in0=kil[:], scalar1=valid,
                            scalar2=None, op0=A.subtract)

            # ---- reduce the loss rows (weights = -live) ----
            mm = ps.tile([P, 8], F32)
            te.matmul(out=mm[0:1, 0:8], lhsT=neglive[:, 0:1],
                      rhs=loss[:, 0:8], start=True, stop=True)
            outs = sb.tile([P, 8], F32)
            v.tensor_copy(out=outs[0:1, :], in_=mm[0:1, :])
            sy.dma_start(out=out_d[:], in_=outs[0:1, 0:8])
            if dbg:
                dt = sb.tile([P, 32], F32)
                v.memset(dt[:], 0.0)
                v.tensor_copy(out=dt[:, 0:5], in_=tg5[:])
                v.tensor_copy(out=dt[:, 5:10], in_=sel5[:])
                v.tensor_copy(out=dt[:, 10:13], in_=iou[:])
                v.tensor_copy(out=dt[:, 13:14], in_=lin[:])
                v.tensor_copy(out=dt[0:N, 14:15], in_=kil[0:N, :])
                v.tensor_copy(out=dt[:, 15:19], in_=gxy[:])
                v.tensor_copy(out=dt[:, 19:21], in_=ijf[:])
                v.tensor_copy(out=dt[:, 21:24], in_=w3[:])
                v.tensor_copy(out=dt[:, 24:32], in_=loss[:])
                sy.dma_start(out=dbg_d[:], in_=dt[:])

    orig = _patch_act_tables()
    try:
        nc.compile()
    finally:
        _unpatch_act_tables(orig)
    return nc


_CACHE = {}


def _get_nc():
    if "nc" not in _CACHE:
        _CACHE["nc"] = _build()
    return _CACHE["nc"]


def _make_cy(y_true_shard):
    """Pack per-core y_true + per-partition index constants: [P, CYW]."""
    cy = np.full((P, CYW), 0.5, np.float32)
    b_local = np.repeat(np.arange(BPC), NGT).astype(np.float32)
    cy[:NGTC, 0:5] = y_true_shard.reshape(NGTC, 5)
    cy[:, 5] = 0.0
    cy[:NGTC, 5] = b_local * PLANE * CHP
    cy[:, 6] = 0.0
    cy[:NGTC, 6] = b_local * 3 * PLANE
    cy[:, 7] = 0.0
    cy[:NGTC, 7] = 1.0
    return np.ascontiguousarray(cy)


def _make_cb(anchors):
    """Data-independent constants, identical on every core: [P, CBW]."""
    cb = np.zeros((P, CBW), np.float32)
    cb[:, 0:6] = np.asarray(anchors, np.float32).reshape(1, 6)
    cb[:, 6:21] = np.tile([1.0, 1.0, 0.0, 0.0, 1.0], NA)[None, :]
    cb[:, 21:101] = np.arange(NCLS, dtype=np.float32)[None, :]
    cb[:, 101:104] = np.array([0.0, PLANE, 2 * PLANE], np.float32)[None, :]
    cb[:, 104:232] = np.eye(P, dtype=np.float32)
    qq = np.arange(P)
    m = (qq[None, :] > qq[:, None]) & (qq[None, :] < NGTC) & (qq[:, None] < NGTC)
    cb[:, 232:360] = m.astype(np.float32)
    return np.ascontiguousarray(cb)


def make_in_maps(x, y_true, anchors):
    x = np.asarray(x, np.float32)
    y_true = np.ascontiguousarray(y_true, np.float32)
    # channel-last layout, padded to 256 channels: [b, gj, gi, ch] so one
    # cell's channels are one aligned contiguous 1KB run in DRAM (layout
    # permutation only, applied while sharding).
    xt = np.zeros((BS, GS, GS, CHP), np.float32)
    xt[..., :NCH] = x.reshape(BS, NCH, GS, GS).transpose(0, 2, 3, 1)
    cb = _make_cb(anchors)
    in_maps = []
    for c in range(NCORES):
        in_maps.append({
            "xf": xt[c * BPC:(c + 1) * BPC].reshape(XF2, 1),
            "cy": _make_cy(y_true[c * BPC:(c + 1) * BPC]),
            "cb": cb,
        })
    return in_maps


def combine_outputs(results):
    cols = np.stack([np.asarray(r["out"], np.float64)[0] for r in results])
    tot = -cols.sum(axis=0)      # device sums are weighted by -live
    n_live = tot[6]
    out = np.empty(6, np.float64)
    out[0:4] = tot[0:4]
    out[4] = tot[5] + (BS * NA * PLANE - n_live) * np.log(np.float64(NCLS))
    out[5] = tot[4] * 25.0
    return out.astype(np.float32)


def run(x, y_true, anchors, trace=False, **kwargs):
    nc = _get_nc()
    res = run_bass_kernel_spmd(nc, make_in_maps(x, y_true, anchors),
                               list(range(NCORES)), trace=trace, **kwargs)
    return combine_outputs(res.results), res


def kernel(x, y_true, anchors):
    out, _ = run(x, y_true, anchors)
    return out

